# revision 2
# baseline (speedup 1.0000x reference)
"""MultiConditionCrossAttention Trainium2 kernel (8 NeuronCores, data-parallel over B).

Math (per batch b):
    q = x @ w_q.T                                  (B, N, 512)
    kv = conditions @ w_kv.T -> k, v               (B, C=16, H=8, hd=64)
    S = einsum('nhd,chd->hnc', q, k) * SCALE       masked softmax over c
    out = einsum('hnc,chd->nhd', attn, v) @ w_proj.T + b_proj

Key restructuring (exact algebra, done per batch on device):
  - Block layouts: K_blk[16h+c, :] = k[c,h,:] placed in head-h's 64-col slice
    (zeros elsewhere); V_blk likewise. Then for all heads at once:
        S_all[n, 16h+c] = q[n] @ K_blk[16h+c]        (block-diag trick)
        out[n]          = attn_all[n] @ V_blk @ w_proj.T + b
  - Weight folding: q only feeds S, and V_blk only feeds the projection, so
        W_s   = K_blk @ (SCALE * w_q)        [128, 512]   (per b)
        W_v2p = V_blk @ w_proj.T + b_proj/8  [128, 512]   (per b)
    using sum_ch attn_all[n, ch] = H = 8 to fold the bias exactly. The whole
    model then collapses to, per 512-token chunk (feature-major on chip):
        S^T  = W_s @ x^T                 (4 f32r matmuls, K=512)
        E    = exp(S^T + mask_bias)      (ACT, per-partition bias)
        Z    = ones_blk.T @ E            (1 matmul -> per-head sums)
        A    = E * recip(Z) broadcast    (sel16 matmul + DVE mul)
        y    = A^T-slices (stationary) @ W_v2p   (4 matmuls -> n-major y)
    x^T comes from 16 PE transposes per chunk. Everything heavy runs f32r
    (1 cycle/row); exact-precision f32 only in the tiny per-b folds.
"""

import os
import numpy as np

import concourse.bass as bass
import concourse.mybir as mybir
import concourse.tile as tile
from concourse import bacc
from concourse.bass_utils import run_bass_kernel_spmd
from concourse.masks import make_identity

F32 = mybir.dt.float32
F32R = mybir.dt.float32r

N_CORES = 8
B, N, D = 16, 8192, 512
C, H, HD = 16, 8, 64
COND_DIM = 256
SCALE = (D // H) ** -0.5
B_PER_CORE = B // N_CORES          # 2
CHUNK = 512                        # tokens per chunk
CHUNKS_PER_B = N // CHUNK          # 16
NEG = -60.0                        # mask bias (exp(-60+s) ~ 0)

_cache = {}


def _build(repeat=1):
    nc = bacc.Bacc("TRN2", target_bir_lowering=False, debug=False,
                   num_devices=N_CORES)

    x_d = nc.dram_tensor("x", [B_PER_CORE, N, D], F32, kind="ExternalInput").ap()
    condT_d = nc.dram_tensor("condT", [B_PER_CORE, COND_DIM, C], F32,
                             kind="ExternalInput").ap()
    wkvT_d = nc.dram_tensor("wkvT", [COND_DIM, 2 * D], F32, kind="ExternalInput").ap()
    wq_d = nc.dram_tensor("wq_scaled", [D, D], F32, kind="ExternalInput").ap()
    wpT_d = nc.dram_tensor("wpT", [D, D], F32, kind="ExternalInput").ap()
    bias8_d = nc.dram_tensor("bias8_rep", [128, D], F32, kind="ExternalInput").ap()
    maskb_d = nc.dram_tensor("mask_bias", [B_PER_CORE, 128, 1], F32,
                             kind="ExternalInput").ap()
    onesb_d = nc.dram_tensor("ones_blk", [128, H], F32, kind="ExternalInput").ap()
    sel16_d = nc.dram_tensor("sel16", [H, 128], F32, kind="ExternalInput").ap()
    y_d = nc.dram_tensor("y", [B_PER_CORE, N, D], F32, kind="ExternalOutput").ap()

    with tile.TileContext(nc) as tc:
        with tc.tile_pool(name="const", bufs=1) as cp:
            ident = cp.tile([128, 128], F32, tag="ident")
            make_identity(nc, ident[:])

            # persistent weights / constants
            wq_sb = []      # [dq-tile][128, 512]: w_q_scaled[dq, ki]
            wp_sb = []      # [dv-tile][128, 512]: w_proj.T[dv, dout]
            for t in range(4):
                w = cp.tile([128, D], F32, tag=f"wq{t}")
                nc.sync.dma_start(w[:], wq_d[t * 128:(t + 1) * 128, :])
                wq_sb.append(w)
                w = cp.tile([128, D], F32, tag=f"wp{t}")
                nc.sync.dma_start(w[:], wpT_d[t * 128:(t + 1) * 128, :])
                wp_sb.append(w)
            wkv_sb = []
            for t in range(2):
                w = cp.tile([128, 2 * D], F32, tag=f"wkv{t}")
                nc.sync.dma_start(w[:], wkvT_d[t * 128:(t + 1) * 128, :])
                wkv_sb.append(w)
            bias8 = cp.tile([128, D], F32, tag="bias8")
            nc.sync.dma_start(bias8[:], bias8_d[:])
            ones_f = cp.tile([128, H], F32, tag="ones_f")
            nc.sync.dma_start(ones_f[:], onesb_d[:])
            ones_r = cp.tile([128, H], F32R, tag="ones_r")
            nc.vector.tensor_copy(ones_r[:], ones_f[:])
            sel_f = cp.tile([H, 128], F32, tag="sel_f")
            nc.sync.dma_start(sel_f[:], sel16_d[:])
            sel_r = cp.tile([H, 128], F32R, tag="sel_r")
            nc.vector.tensor_copy(sel_r[:], sel_f[:])
            maskb = []
            for b in range(B_PER_CORE):
                m = cp.tile([128, 1], F32, tag=f"maskb{b}")
                nc.sync.dma_start(m[:], maskb_d[b])
                maskb.append(m)

            # ---------------- per-b folded weights ----------------
            wsT = []    # [b] -> [128, 4, 128] f32r : W_s.T tiles (ki-tile, ch)
            wv2p = []   # [b] -> [128, 512] f32r : V_blk @ w_proj.T + b/8
            with (
                tc.tile_pool(name="pre_sb", bufs=1) as pp,
                tc.tile_pool(name="pre_ps", bufs=1, space="PSUM") as pps,
            ):
                for b in range(B_PER_CORE):
                    ct = []
                    for t in range(2):
                        cti = pp.tile([128, C], F32, tag=f"ct{t}")
                        nc.sync.dma_start(cti[:], condT_d[b, t * 128:(t + 1) * 128, :])
                        ct.append(cti)
                    # kv projection: k/v [C=16, 512] (c, h*64+d)
                    k_ps = pps.tile([C, D], F32, tag="k_ps")
                    v_ps = pps.tile([C, D], F32, tag="v_ps")
                    for t in range(2):
                        nc.tensor.matmul(k_ps[:], ct[t][:], wkv_sb[t][:, 0:D],
                                         start=(t == 0), stop=(t == 1))
                        nc.tensor.matmul(v_ps[:], ct[t][:], wkv_sb[t][:, D:2 * D],
                                         start=(t == 0), stop=(t == 1))
                    k_sb = pp.tile([C, D], F32, tag="k_sb")
                    nc.any.tensor_copy(k_sb[:], k_ps[:])
                    v_sb = pp.tile([C, D], F32, tag="v_sb")
                    nc.any.tensor_copy(v_sb[:], v_ps[:])

                    # build K_blk.T / V_blk.T as [128, 4, 128] (d-tile, ch) f32
                    kblkT = pp.tile([128, 4, 128], F32, tag="kblkT")
                    nc.vector.memset(kblkT[:], 0.0)
                    vblkT = pp.tile([128, 4, 128], F32, tag="vblkT")
                    nc.vector.memset(vblkT[:], 0.0)
                    for src_sb, dst in ((k_sb, kblkT), (v_sb, vblkT)):
                        for h in range(H):
                            tp = pps.tile([HD, C], F32, tag="tp")
                            nc.tensor.transpose(tp[:], src_sb[:, h * HD:(h + 1) * HD],
                                                ident[:C, :C])
                            st = pp.tile([HD, C], F32, tag="st")
                            nc.any.tensor_copy(st[:], tp[:])
                            nc.gpsimd.dma_start(
                                dst[(h % 2) * HD:(h % 2 + 1) * HD, h // 2,
                                    C * h:C * (h + 1)],
                                st[:])

                    # W_s.T [ki, ch] = sum_dq wq_scaled[dq, ki] * K_blk.T[dq, ch]
                    ws = cp.tile([128, 4, 128], F32R, tag=f"wsT{b}")
                    for kit in range(4):
                        ws_ps = pps.tile([128, 128], F32, tag="ws_ps")
                        for dqt in range(4):
                            nc.tensor.matmul(
                                ws_ps[:],
                                wq_sb[dqt][:, kit * 128:(kit + 1) * 128],
                                kblkT[:, dqt, :],
                                start=(dqt == 0), stop=(dqt == 3))
                        nc.any.tensor_copy(ws[:, kit, :], ws_ps[:])
                    wsT.append(ws)

                    # W_v2p [ch, dout] = sum_dv V_blk.T[dv, ch].T @ wpT[dv, dout] + b/8
                    wv_ps = pps.tile([128, D], F32, tag="wv_ps")
                    for dvt in range(4):
                        nc.tensor.matmul(wv_ps[:], vblkT[:, dvt, :], wp_sb[dvt][:],
                                         start=(dvt == 0), stop=(dvt == 3))
                    wv = cp.tile([128, D], F32R, tag=f"wv2p{b}")
                    nc.vector.tensor_add(wv[:], wv_ps[:], bias8[:])
                    wv2p.append(wv)

            # ---------------- main loop ----------------
            with (
                tc.tile_pool(name="m_sb", bufs=2) as mp,
                tc.tile_pool(name="ps_xt", bufs=2, space="PSUM") as ps_xt,
                tc.tile_pool(name="ps_s", bufs=2, space="PSUM") as ps_s,
                tc.tile_pool(name="ps_z", bufs=1, space="PSUM") as ps_z,
                tc.tile_pool(name="ps_zb", bufs=1, space="PSUM") as ps_zb,
                tc.tile_pool(name="ps_y", bufs=2, space="PSUM") as ps_y,
            ):
                for rep in range(repeat):
                    for b in range(B_PER_CORE):
                        for ci in range(CHUNKS_PER_B):
                            n0 = ci * CHUNK
                            x_src = x_d[b, n0:n0 + CHUNK, :].rearrange(
                                "(g p) k -> p g k", p=128)
                            x_sb = mp.tile([128, 4, D], F32, tag="x_sb")
                            nc.sync.dma_start(x_sb[:], x_src)

                            # transpose x -> x^T tiles [k-tile][128, 512] f32r
                            xT = mp.tile([128, 4, CHUNK], F32R, tag="xT")
                            for kt in range(4):
                                xt_ps = ps_xt.tile([128, CHUNK], F32, tag="xt_ps")
                                for g in range(4):
                                    nc.tensor.transpose(
                                        xt_ps[:, g * 128:(g + 1) * 128],
                                        x_sb[:, g, kt * 128:(kt + 1) * 128],
                                        ident[:])
                                nc.any.tensor_copy(xT[:, kt, :], xt_ps[:])

                            # S^T = W_s @ x^T  [128 ch, 512 n]
                            s_ps = ps_s.tile([128, CHUNK], F32, tag="s_ps")
                            for kt in range(4):
                                nc.tensor.matmul(s_ps[:], wsT[b][:, kt, :],
                                                 xT[:, kt, :],
                                                 start=(kt == 0), stop=(kt == 3))

                            # E = exp(S + mask_bias)
                            e_r = mp.tile([128, CHUNK], F32R, tag="e_r")
                            nc.scalar.activation(e_r[:], s_ps[:],
                                                 mybir.ActivationFunctionType.Exp,
                                                 bias=maskb[b][:], scale=1.0)

                            # Z[h, n] = sum_{c in h} E ; recip; broadcast to [128, n]
                            z_ps = ps_z.tile([H, CHUNK], F32, tag="z_ps")
                            nc.tensor.matmul(z_ps[:], ones_r[:], e_r[:],
                                             start=True, stop=True)
                            rz = mp.tile([H, CHUNK], F32, tag="rz")
                            nc.vector.reciprocal_approx_fast(rz[:], z_ps[:])
                            rz_r = mp.tile([H, CHUNK], F32R, tag="rz_r")
                            nc.vector.tensor_copy(rz_r[:], rz[:])
                            zb_ps = ps_zb.tile([128, CHUNK], F32, tag="zb_ps")
                            nc.tensor.matmul(zb_ps[:], sel_r[:], rz_r[:],
                                             start=True, stop=True)

                            # A = E * Zb  (normalized attention, f32r)
                            a_r = mp.tile([128, CHUNK], F32R, tag="a_r")
                            nc.vector.tensor_mul(a_r[:], e_r[:], zb_ps[:])

                            # y[n-sub g] = A[:, g].T @ W_v2p  -> [128 n, 512 dout]
                            y_sb = mp.tile([128, 4, D], F32, tag="y_sb")
                            for g in range(4):
                                y_ps = ps_y.tile([128, D], F32, tag="y_ps")
                                nc.tensor.matmul(y_ps[:],
                                                 a_r[:, g * 128:(g + 1) * 128],
                                                 wv2p[b][:],
                                                 start=True, stop=True)
                                nc.any.tensor_copy(y_sb[:, g, :], y_ps[:])

                            y_dst = y_d[b, n0:n0 + CHUNK, :].rearrange(
                                "(g p) k -> p g k", p=128)
                            nc.scalar.dma_start(y_dst, y_sb[:])

    nc.compile()
    return nc


def _prep_inputs(x, conditions, condition_mask, w_q, w_kv, w_proj, b_proj):
    """Host-side marshalling: shard over B, transpose/scale small weights."""
    x = np.ascontiguousarray(x, dtype=np.float32)
    conditions = np.asarray(conditions, dtype=np.float32)
    condition_mask = np.asarray(condition_mask)
    w_q = np.asarray(w_q, dtype=np.float32)
    w_kv = np.asarray(w_kv, dtype=np.float32)
    w_proj = np.asarray(w_proj, dtype=np.float32)
    b_proj = np.asarray(b_proj, dtype=np.float32)

    wq_scaled = np.ascontiguousarray(w_q * SCALE)            # [dq, ki]
    wkvT = np.ascontiguousarray(w_kv.T)                      # [256, 1024]
    wpT = np.ascontiguousarray(w_proj.T)                     # [dv, dout]
    bias8_rep = np.ascontiguousarray(
        np.tile((b_proj / H)[None, :], (128, 1)).astype(np.float32))
    onesb = np.zeros((128, H), dtype=np.float32)
    sel16 = np.zeros((H, 128), dtype=np.float32)
    for h in range(H):
        onesb[h * C:(h + 1) * C, h] = 1.0
        sel16[h, h * C:(h + 1) * C] = 1.0

    in_maps = []
    for core in range(N_CORES):
        b0 = core * B_PER_CORE
        condT = np.ascontiguousarray(
            np.transpose(conditions[b0:b0 + B_PER_CORE], (0, 2, 1)))
        mb = np.zeros((B_PER_CORE, 128, 1), dtype=np.float32)
        for b in range(B_PER_CORE):
            m = condition_mask[b0 + b].astype(bool)          # [16]
            col = np.where(np.tile(m, H), 0.0, NEG).astype(np.float32)
            mb[b, :, 0] = col
        in_maps.append(dict(
            x=np.ascontiguousarray(x[b0:b0 + B_PER_CORE]),
            condT=condT,
            wkvT=wkvT,
            wq_scaled=wq_scaled,
            wpT=wpT,
            bias8_rep=bias8_rep,
            mask_bias=mb,
            ones_blk=onesb,
            sel16=sel16,
        ))
    return in_maps


def kernel(x, conditions, condition_mask, w_q, w_kv, w_proj, b_proj):
    repeat = int(os.environ.get("MCCA_REPEAT", "1"))
    key = ("nc", repeat)
    if key not in _cache:
        _cache[key] = _build(repeat=repeat)
    nc = _cache[key]
    in_maps = _prep_inputs(x, conditions, condition_mask, w_q, w_kv,
                           w_proj, b_proj)
    res = run_bass_kernel_spmd(nc, in_maps, core_ids=list(range(N_CORES)))
    y = np.concatenate([r["y"] for r in res.results], axis=0)  # [16, 8192, 512]
    return np.ascontiguousarray(y.astype(np.float32))


# revision 25
# speedup vs baseline: 52579.9262x; 52579.9262x over previous
"""MultiConditionCrossAttention Trainium2 kernel (8 NeuronCores, data-parallel over B).

Math (per batch b):
    q = x @ w_q.T                                  (B, N, 512)
    kv = conditions @ w_kv.T -> k, v               (B, C=16, H=8, hd=64)
    S = einsum('nhd,chd->hnc', q, k) * SCALE       masked softmax over c
    out = einsum('hnc,chd->nhd', attn, v) @ w_proj.T + b_proj

Key restructuring (exact algebra, done per batch on device):
  - Block layouts: K_blk[16h+c, :] = k[c,h,:] placed in head-h's 64-col slice
    (zeros elsewhere); V_blk likewise. Then for all heads at once:
        S_all[n, 16h+c] = q[n] @ K_blk[16h+c]        (block-diag trick)
        out[n]          = attn_all[n] @ V_blk @ w_proj.T + b
  - Weight folding: q only feeds S, and V_blk only feeds the projection, so
        W_s   = K_blk @ (SCALE * w_q)        [128, 512]   (per b)
        W_v2p = V_blk @ w_proj.T + b_proj/8  [128, 512]   (per b)
    using sum_ch attn_all[n, ch] = H = 8 to fold the bias exactly. The whole
    model then collapses to, per 512-token chunk (feature-major on chip):
        S^T  = W_s @ x^T                 (4 f32r matmuls, K=512)
        E    = exp(S^T + mask_bias)      (ACT, per-partition bias)
        Z    = ones_blk.T @ E            (1 matmul -> per-head sums)
        A    = E * recip(Z) broadcast    (sel16 matmul + DVE mul)
        y    = A^T-slices (stationary) @ W_v2p   (4 matmuls -> n-major y)
    x^T comes from 16 PE transposes per chunk. Everything heavy runs f32r
    (1 cycle/row); exact-precision f32 only in the tiny per-b folds.
"""

import os
import numpy as np

import concourse.bass as bass
import concourse.mybir as mybir
import concourse.tile as tile
from concourse import bacc
from concourse.bass_utils import run_bass_kernel_spmd
from concourse.masks import make_identity

F32 = mybir.dt.float32
F32R = mybir.dt.float32r

N_CORES = 8
B, N, D = 16, 8192, 512
C, H, HD = 16, 8, 64
COND_DIM = 256
SCALE = (D // H) ** -0.5
B_PER_CORE = B // N_CORES          # 2
CHUNK = 512                        # tokens per chunk
CHUNKS_PER_B = N // CHUNK          # 16
NEG = -60.0                        # mask bias (exp(-60+s) ~ 0)

_cache = {}


def _build(repeat=1, bufs_x=6, bufs_ysb=4, bufs_sm=3, bufs_xt=2, bufs_s=1,
           bufs_zb=1, bufs_y=4, skip=()):
    nc = bacc.Bacc("TRN2", target_bir_lowering=False, debug=False,
                   num_devices=N_CORES)

    x_d = nc.dram_tensor("x", [B_PER_CORE, N, D], F32, kind="ExternalInput").ap()
    condT_d = nc.dram_tensor("condT", [B_PER_CORE, COND_DIM, C], F32,
                             kind="ExternalInput").ap()
    wkvT_d = nc.dram_tensor("wkvT", [COND_DIM, 2 * D], F32, kind="ExternalInput").ap()
    wq_d = nc.dram_tensor("wq_scaled", [D, D], F32, kind="ExternalInput").ap()
    wpT_d = nc.dram_tensor("wpT", [D, D], F32, kind="ExternalInput").ap()
    bias8_d = nc.dram_tensor("bias8_rep", [128, D], F32, kind="ExternalInput").ap()
    maskb_d = nc.dram_tensor("mask_bias", [B_PER_CORE, 128, 1], F32,
                             kind="ExternalInput").ap()
    blk16_d = nc.dram_tensor("blk16", [128, 128], F32, kind="ExternalInput").ap()
    y_d = nc.dram_tensor("y", [B_PER_CORE, N, D], F32, kind="ExternalOutput").ap()

    from contextlib import ExitStack
    with tile.TileContext(nc) as tc:
        with ExitStack() as stack:
            cp = stack.enter_context(tc.tile_pool(name="const", bufs=1))
            ident = cp.tile([128, 128], F32, tag="ident")
            make_identity(nc, ident[:])

            # conditions first: the whole preamble fold chain hangs off them
            cond_sb = []
            for b in range(B_PER_CORE):
                cts = []
                for t in range(2):
                    cti = cp.tile([128, C], F32, tag=f"ct{b}_{t}")
                    nc.sync.dma_start(cti[:], condT_d[b, t * 128:(t + 1) * 128, :])
                    cts.append(cti)
                cond_sb.append(cts)
            # persistent weights / constants, in fold-chain dependency order:
            # wkv feeds the kT matmuls first, then wq (fold-S), then wp (fold-P)
            wkv_sb = []
            for t in range(2):
                w = cp.tile([128, 2 * D], F32, tag=f"wkv{t}")
                nc.sync.dma_start(w[:], wkvT_d[t * 128:(t + 1) * 128, :])
                wkv_sb.append(w)
            wq_sb = []      # [dq-tile][128, 512]: w_q_scaled[dq, ki]
            wp_sb = []      # [dv-tile][128, 512]: w_proj.T[dv, dout]
            for t in range(4):
                w = cp.tile([128, D], F32, tag=f"wq{t}")
                nc.sync.dma_start(w[:], wq_d[t * 128:(t + 1) * 128, :])
                wq_sb.append(w)
            for t in range(4):
                w = cp.tile([128, D], F32, tag=f"wp{t}")
                nc.sync.dma_start(w[:], wpT_d[t * 128:(t + 1) * 128, :])
                wp_sb.append(w)
            bias8 = cp.tile([128, D], F32, tag="bias8")
            nc.sync.dma_start(bias8[:], bias8_d[:])
            blk16_f = cp.tile([128, 128], F32, tag="blk16_f")
            nc.sync.dma_start(blk16_f[:], blk16_d[:])
            blk16_r = cp.tile([128, 128], F32R, tag="blk16_r")
            nc.vector.tensor_copy(blk16_r[:], blk16_f[:])
            maskb = []
            for b in range(B_PER_CORE):
                m = cp.tile([128, 1], F32, tag=f"maskb{b}")
                nc.sync.dma_start(m[:], maskb_d[b])
                maskb.append(m)

            # ---------------- per-b folded weights ----------------
            wsT = []    # [b] -> [128, 4, 128] f32r : W_s.T tiles (ki-tile, ch)
            wv2p = []   # [b] -> [128, 512] f32r : V_blk @ w_proj.T + b/8
            # pre_sb spans the whole kernel so its SBUF is not aliased by
            # the main-loop pools (aliasing would serialize the first
            # x-loads behind the preamble compute chain)
            pp = stack.enter_context(tc.tile_pool(name="pre_sb", bufs=2))
            with tc.tile_pool(name="pre_ps", bufs=1, space="PSUM") as pps:
                for b in range(B_PER_CORE):
                    ct = cond_sb[b]
                    # kv projection, feature-major directly:
                    # kT[dk-tile][128, C] = (wkvT slice).T @ condT  = w_kv @ cond^T
                    # Then K_blk.T [128, 4, 128] (dq-in-tile, dq-tile, ch) is
                    # assembled with two same-partition block copies per tile
                    # (head 2t -> rows 0:64 cols 16(2t..), head 2t+1 -> rows
                    # 64:128 cols 16(2t+1..)); zeros elsewhere.
                    kblkT = pp.tile([128, 4, 128], F32, tag="kblkT")
                    nc.vector.memset(kblkT[:], 0.0)
                    vblkT = pp.tile([128, 4, 128], F32, tag="vblkT")
                    nc.vector.memset(vblkT[:], 0.0)
                    for off, dst in ((0, kblkT), (D, vblkT)):
                        for t in range(4):
                            kt_ps = pps.tile([128, C], F32, tag="kt_ps")
                            for u in range(2):
                                nc.tensor.matmul(
                                    kt_ps[:],
                                    wkv_sb[u][:, off + t * 128:off + (t + 1) * 128],
                                    ct[u][:],
                                    start=(u == 0), stop=(u == 1))
                            for half in range(2):
                                h = 2 * t + half
                                nc.vector.tensor_copy(
                                    dst[half * HD:(half + 1) * HD, t,
                                        C * h:C * (h + 1)],
                                    kt_ps[half * HD:(half + 1) * HD, :])

                    # W_s.T [ki, ch] = sum_dq wq_scaled[dq, ki] * K_blk.T[dq, ch]
                    ws = cp.tile([128, 4, 128], F32R, tag=f"wsT{b}")
                    for kit in range(4):
                        ws_ps = pps.tile([128, 128], F32, tag="ws_ps")
                        for dqt in range(4):
                            nc.tensor.matmul(
                                ws_ps[:],
                                wq_sb[dqt][:, kit * 128:(kit + 1) * 128],
                                kblkT[:, dqt, :],
                                start=(dqt == 0), stop=(dqt == 3))
                        nc.vector.tensor_copy(ws[:, kit, :], ws_ps[:])
                    wsT.append(ws)

                    # W_v2p [ch, dout] = sum_dv V_blk.T[dv, ch].T @ wpT[dv, dout] + b/8
                    wv_ps = pps.tile([128, D], F32, tag="wv_ps")
                    for dvt in range(4):
                        nc.tensor.matmul(wv_ps[:], vblkT[:, dvt, :], wp_sb[dvt][:],
                                         start=(dvt == 0), stop=(dvt == 3))
                    wv = cp.tile([128, D], F32R, tag=f"wv2p{b}")
                    nc.vector.tensor_add(wv[:], wv_ps[:], bias8[:])
                    wv2p.append(wv)

            # ---------------- main loop ----------------
            with (
                tc.tile_pool(name="m_x", bufs=bufs_x) as mp_x,
                tc.tile_pool(name="m_ys", bufs=bufs_ysb) as mp_y,
                tc.tile_pool(name="m_sm", bufs=bufs_sm) as mp_s,
                # zb/y first: they alias the (closed) preamble PSUM banks and
                # are used late in each chunk, so chunk-0 transposes (xt) get
                # fresh banks and need not wait for the preamble to drain
                tc.tile_pool(name="ps_zb", bufs=bufs_zb, space="PSUM") as ps_zb,
                tc.tile_pool(name="ps_y", bufs=bufs_y, space="PSUM") as ps_y,
                tc.tile_pool(name="ps_xt", bufs=bufs_xt, space="PSUM") as ps_xt,
                tc.tile_pool(name="ps_s", bufs=bufs_s, space="PSUM") as ps_s,
            ):
                from contextlib import nullcontext
                rep_ctx = tc.For_i(0, repeat, 1) if repeat > 1 else nullcontext()
                with rep_ctx:
                    for b in range(B_PER_CORE):
                        for ci in range(CHUNKS_PER_B):
                            n0 = ci * CHUNK
                            x_src = x_d[b, n0:n0 + CHUNK, :].rearrange(
                                "(g p) k -> p g k", p=128)
                            x_sb = mp_x.tile([128, 4, D], F32, tag="x_sb")
                            if "load" not in skip:
                                nc.sync.dma_start(x_sb[:, 0:2, :], x_src[:, 0:2, :])
                                nc.sync.dma_start(x_sb[:, 2:4, :], x_src[:, 2:4, :])
                            else:
                                nc.vector.memset(x_sb[:, 0, 0:4], 0.0)

                            # transpose x -> x^T tiles [k-tile][128, 512] f32r
                            xT = mp_x.tile([128, 4, CHUNK], F32R, tag="xT")
                            if "transpose" not in skip:
                                for kt in range(4):
                                    xt_ps = ps_xt.tile([128, CHUNK], F32, tag="xt_ps")
                                    for g in range(4):
                                        nc.tensor.transpose(
                                            xt_ps[:, g * 128:(g + 1) * 128],
                                            x_sb[:, g, kt * 128:(kt + 1) * 128],
                                            ident[:])
                                    if kt % 2 == 0:
                                        nc.vector.tensor_copy(xT[:, kt, :], xt_ps[:])
                                    else:
                                        nc.scalar.copy(xT[:, kt, :], xt_ps[:])
                            else:
                                nc.vector.tensor_copy(xT[:, 0, 0:4], x_sb[:, 0, 0:4])

                            # S^T = W_s @ x^T  [128 ch, 512 n]
                            s_ps = ps_s.tile([128, CHUNK], F32, tag="s_ps")
                            for kt in range(4):
                                nc.tensor.matmul(s_ps[:], wsT[b][:, kt, :],
                                                 xT[:, kt, :],
                                                 start=(kt == 0), stop=(kt == 3))

                            # E = exp(S + mask_bias)
                            e_r = mp_s.tile([128, CHUNK], F32R, tag="e_r")
                            nc.scalar.activation(e_r[:], s_ps[:],
                                                 mybir.ActivationFunctionType.Exp,
                                                 bias=maskb[b][:], scale=1.0)

                            a_r = mp_s.tile([128, CHUNK], F32R, tag="a_r")
                            if "softmax" not in skip:
                                # Zrep[ch, n] = per-head sum of E, replicated
                                zb_ps = ps_zb.tile([128, CHUNK], F32, tag="zb_ps")
                                nc.tensor.matmul(zb_ps[:], blk16_r[:], e_r[:],
                                                 start=True, stop=True)
                                rzb = mp_s.tile([128, CHUNK], F32, tag="rzb")
                                nc.vector.reciprocal_approx_fast(rzb[:], zb_ps[:])
                                # A = E * recip(Zrep)  (normalized attention, f32r)
                                nc.vector.tensor_mul(a_r[:], e_r[:], rzb[:])
                            else:
                                nc.vector.tensor_copy(a_r[:], e_r[:])

                            # y[n-sub g] = A[:, g].T @ W_v2p  -> [128 n, 512 dout]
                            y_sb = mp_y.tile([128, 4, D], F32, tag="y_sb")
                            for g in range(4):
                                y_ps = ps_y.tile([128, D], F32, tag="y_ps")
                                nc.tensor.matmul(y_ps[:],
                                                 a_r[:, g * 128:(g + 1) * 128],
                                                 wv2p[b][:],
                                                 start=True, stop=True)
                                if g % 2 == 0:
                                    nc.scalar.copy(y_sb[:, g, :], y_ps[:])
                                else:
                                    nc.vector.tensor_copy(y_sb[:, g, :], y_ps[:])

                            if "store" not in skip:
                                y_dst = y_d[b, n0:n0 + CHUNK, :].rearrange(
                                    "(g p) k -> p g k", p=128)
                                nc.scalar.dma_start(y_dst, y_sb[:])

    nc.compile()
    return nc


def _prep_inputs(x, conditions, condition_mask, w_q, w_kv, w_proj, b_proj):
    """Host-side marshalling: shard over B, transpose/scale small weights."""
    x = np.ascontiguousarray(x, dtype=np.float32)
    conditions = np.asarray(conditions, dtype=np.float32)
    condition_mask = np.asarray(condition_mask)
    w_q = np.asarray(w_q, dtype=np.float32)
    w_kv = np.asarray(w_kv, dtype=np.float32)
    w_proj = np.asarray(w_proj, dtype=np.float32)
    b_proj = np.asarray(b_proj, dtype=np.float32)

    wq_scaled = np.ascontiguousarray(w_q * SCALE)            # [dq, ki]
    wkvT = np.ascontiguousarray(w_kv.T)                      # [256, 1024]
    wpT = np.ascontiguousarray(w_proj.T)                     # [dv, dout]
    bias8_rep = np.ascontiguousarray(
        np.tile((b_proj / H)[None, :], (128, 1)).astype(np.float32))
    blk16 = np.zeros((128, 128), dtype=np.float32)
    for h in range(H):
        blk16[h * C:(h + 1) * C, h * C:(h + 1) * C] = 1.0

    in_maps = []
    for core in range(N_CORES):
        b0 = core * B_PER_CORE
        condT = np.ascontiguousarray(
            np.transpose(conditions[b0:b0 + B_PER_CORE], (0, 2, 1)))
        mb = np.zeros((B_PER_CORE, 128, 1), dtype=np.float32)
        for b in range(B_PER_CORE):
            m = condition_mask[b0 + b].astype(bool)          # [16]
            col = np.where(np.tile(m, H), 0.0, NEG).astype(np.float32)
            mb[b, :, 0] = col
        in_maps.append(dict(
            x=np.ascontiguousarray(x[b0:b0 + B_PER_CORE]),
            condT=condT,
            wkvT=wkvT,
            wq_scaled=wq_scaled,
            wpT=wpT,
            bias8_rep=bias8_rep,
            mask_bias=mb,
            blk16=blk16,
        ))
    return in_maps


def kernel(x, conditions, condition_mask, w_q, w_kv, w_proj, b_proj):
    repeat = int(os.environ.get("MCCA_REPEAT", "1"))
    key = ("nc", repeat)
    if key not in _cache:
        _cache[key] = _build(repeat=repeat)
    nc = _cache[key]
    in_maps = _prep_inputs(x, conditions, condition_mask, w_q, w_kv,
                           w_proj, b_proj)
    res = run_bass_kernel_spmd(nc, in_maps, core_ids=list(range(N_CORES)))
    y = np.concatenate([r["y"] for r in res.results], axis=0)  # [16, 8192, 512]
    return np.ascontiguousarray(y.astype(np.float32))


# revision 26
# speedup vs baseline: 54103.0448x; 1.0290x over previous
"""MultiConditionCrossAttention Trainium2 kernel (8 NeuronCores, data-parallel over B).

Math (per batch b):
    q = x @ w_q.T                                  (B, N, 512)
    kv = conditions @ w_kv.T -> k, v               (B, C=16, H=8, hd=64)
    S = einsum('nhd,chd->hnc', q, k) * SCALE       masked softmax over c
    out = einsum('hnc,chd->nhd', attn, v) @ w_proj.T + b_proj

Key restructuring (exact algebra, done per batch on device):
  - Block layouts: K_blk[16h+c, :] = k[c,h,:] placed in head-h's 64-col slice
    (zeros elsewhere); V_blk likewise. Then for all heads at once:
        S_all[n, 16h+c] = q[n] @ K_blk[16h+c]        (block-diag trick)
        out[n]          = attn_all[n] @ V_blk @ w_proj.T + b
  - Weight folding: q only feeds S, and V_blk only feeds the projection, so
        W_s   = K_blk @ (SCALE * w_q)        [128, 512]   (per b)
        W_v2p = V_blk @ w_proj.T + b_proj/8  [128, 512]   (per b)
    using sum_ch attn_all[n, ch] = H = 8 to fold the bias exactly. The whole
    model then collapses to, per 512-token chunk (feature-major on chip):
        S^T  = W_s @ x^T                  (4 f32r matmuls, K=512)
        E    = exp(S^T + mask_bias)       (ACT, per-partition bias)
        Zrep = blk16.T @ E                (1 matmul -> per-head sums, replicated)
        A    = E * recip_approx(Zrep)     (DVE)
        y    = A^T n-slices (stationary) @ W_v2p  (4 matmuls -> n-major y)
    x^T comes from 16 PE transposes per chunk (the only transposes in the
    kernel; kv projection emits feature-major directly). Everything heavy
    runs f32r (1 PE cycle/row at >=256 moving cols, ~1e-4 rel precision);
    exact f32 only in the tiny per-b folds.

    Measured on 8x trn2 NeuronCores: ~205-212 us/core one-shot (cost model /
    For_i-loop dilution), vs a ~188 us pure x+y HBM roofline (67 MB/core).
    End-to-end relative error vs the fp32 jax reference: 2.2e-4.
"""

import os
import numpy as np

import concourse.mybir as mybir
import concourse.tile as tile
from concourse import bacc
from concourse.bass_utils import run_bass_kernel_spmd
from concourse.masks import make_identity

F32 = mybir.dt.float32
F32R = mybir.dt.float32r

N_CORES = 8
B, N, D = 16, 8192, 512
C, H, HD = 16, 8, 64
COND_DIM = 256
SCALE = (D // H) ** -0.5
B_PER_CORE = B // N_CORES          # 2
CHUNK = 512                        # tokens per chunk
CHUNKS_PER_B = N // CHUNK          # 16
NEG = -60.0                        # mask bias (exp(-60+s) ~ 0)

_cache = {}


def _build(repeat=1, bufs_x=6, bufs_ysb=4, bufs_sm=3, bufs_xt=2, bufs_s=1,
           bufs_zb=1, bufs_y=4, skip=()):
    nc = bacc.Bacc("TRN2", target_bir_lowering=False, debug=False,
                   num_devices=N_CORES)

    x_d = nc.dram_tensor("x", [B_PER_CORE, N, D], F32, kind="ExternalInput").ap()
    condT_d = nc.dram_tensor("condT", [B_PER_CORE, COND_DIM, C], F32,
                             kind="ExternalInput").ap()
    wkvT_d = nc.dram_tensor("wkvT", [COND_DIM, 2 * D], F32, kind="ExternalInput").ap()
    wq_d = nc.dram_tensor("wq_scaled", [D, D], F32, kind="ExternalInput").ap()
    wpT_d = nc.dram_tensor("wpT", [D, D], F32, kind="ExternalInput").ap()
    bias8_d = nc.dram_tensor("bias8_rep", [128, D], F32, kind="ExternalInput").ap()
    maskb_d = nc.dram_tensor("mask_bias", [B_PER_CORE, 128, 1], F32,
                             kind="ExternalInput").ap()
    blk16_d = nc.dram_tensor("blk16", [128, 128], F32, kind="ExternalInput").ap()
    y_d = nc.dram_tensor("y", [B_PER_CORE, N, D], F32, kind="ExternalOutput").ap()

    from contextlib import ExitStack
    with tile.TileContext(nc) as tc:
        with ExitStack() as stack:
            cp = stack.enter_context(tc.tile_pool(name="const", bufs=1))
            ident = cp.tile([128, 128], F32, tag="ident")
            make_identity(nc, ident[:])

            # conditions first: the whole preamble fold chain hangs off them
            cond_sb = []
            for b in range(B_PER_CORE):
                cts = []
                for t in range(2):
                    cti = cp.tile([128, C], F32, tag=f"ct{b}_{t}")
                    nc.sync.dma_start(cti[:], condT_d[b, t * 128:(t + 1) * 128, :])
                    cts.append(cti)
                cond_sb.append(cts)
            # persistent weights / constants, in fold-chain dependency order:
            # wkv feeds the kT matmuls first, then wq (fold-S), then wp (fold-P)
            wkv_sb = []
            for t in range(2):
                w = cp.tile([128, 2 * D], F32, tag=f"wkv{t}")
                nc.sync.dma_start(w[:], wkvT_d[t * 128:(t + 1) * 128, :])
                wkv_sb.append(w)
            wq_sb = []      # [dq-tile][128, 512]: w_q_scaled[dq, ki]
            wp_sb = []      # [dv-tile][128, 512]: w_proj.T[dv, dout]
            for t in range(4):
                w = cp.tile([128, D], F32, tag=f"wq{t}")
                nc.sync.dma_start(w[:], wq_d[t * 128:(t + 1) * 128, :])
                wq_sb.append(w)
            for t in range(4):
                w = cp.tile([128, D], F32, tag=f"wp{t}")
                nc.sync.dma_start(w[:], wpT_d[t * 128:(t + 1) * 128, :])
                wp_sb.append(w)
            bias8 = cp.tile([128, D], F32, tag="bias8")
            nc.sync.dma_start(bias8[:], bias8_d[:])
            blk16_f = cp.tile([128, 128], F32, tag="blk16_f")
            nc.sync.dma_start(blk16_f[:], blk16_d[:])
            blk16_r = cp.tile([128, 128], F32R, tag="blk16_r")
            nc.vector.tensor_copy(blk16_r[:], blk16_f[:])
            maskb = []
            for b in range(B_PER_CORE):
                m = cp.tile([128, 1], F32, tag=f"maskb{b}")
                nc.sync.dma_start(m[:], maskb_d[b])
                maskb.append(m)

            # ---------------- per-b folded weights ----------------
            wsT = []    # [b] -> [128, 4, 128] f32r : W_s.T tiles (ki-tile, ch)
            wv2p = []   # [b] -> [128, 512] f32r : V_blk @ w_proj.T + b/8
            # pre_sb spans the whole kernel so its SBUF is not aliased by
            # the main-loop pools (aliasing would serialize the first
            # x-loads behind the preamble compute chain)
            pp = stack.enter_context(tc.tile_pool(name="pre_sb", bufs=2))
            with tc.tile_pool(name="pre_ps", bufs=1, space="PSUM") as pps:
                for b in range(B_PER_CORE):
                    ct = cond_sb[b]
                    # kv projection, feature-major directly:
                    # kT[dk-tile][128, C] = (wkvT slice).T @ condT  = w_kv @ cond^T
                    # Then K_blk.T [128, 4, 128] (dq-in-tile, dq-tile, ch) is
                    # assembled with two same-partition block copies per tile
                    # (head 2t -> rows 0:64 cols 16(2t..), head 2t+1 -> rows
                    # 64:128 cols 16(2t+1..)); zeros elsewhere.
                    kblkT = pp.tile([128, 4, 128], F32, tag="kblkT")
                    nc.vector.memset(kblkT[:], 0.0)
                    vblkT = pp.tile([128, 4, 128], F32, tag="vblkT")
                    nc.vector.memset(vblkT[:], 0.0)
                    for off, dst in ((0, kblkT), (D, vblkT)):
                        for t in range(4):
                            kt_ps = pps.tile([128, C], F32, tag="kt_ps")
                            for u in range(2):
                                nc.tensor.matmul(
                                    kt_ps[:],
                                    wkv_sb[u][:, off + t * 128:off + (t + 1) * 128],
                                    ct[u][:],
                                    start=(u == 0), stop=(u == 1))
                            for half in range(2):
                                h = 2 * t + half
                                nc.vector.tensor_copy(
                                    dst[half * HD:(half + 1) * HD, t,
                                        C * h:C * (h + 1)],
                                    kt_ps[half * HD:(half + 1) * HD, :])

                    # W_s.T [ki, ch] = sum_dq wq_scaled[dq, ki] * K_blk.T[dq, ch]
                    ws = cp.tile([128, 4, 128], F32R, tag=f"wsT{b}")
                    for kit in range(4):
                        ws_ps = pps.tile([128, 128], F32, tag="ws_ps")
                        for dqt in range(4):
                            nc.tensor.matmul(
                                ws_ps[:],
                                wq_sb[dqt][:, kit * 128:(kit + 1) * 128],
                                kblkT[:, dqt, :],
                                start=(dqt == 0), stop=(dqt == 3))
                        nc.vector.tensor_copy(ws[:, kit, :], ws_ps[:])
                    wsT.append(ws)

                    # W_v2p [ch, dout] = sum_dv V_blk.T[dv, ch].T @ wpT[dv, dout] + b/8
                    wv_ps = pps.tile([128, D], F32, tag="wv_ps")
                    for dvt in range(4):
                        nc.tensor.matmul(wv_ps[:], vblkT[:, dvt, :], wp_sb[dvt][:],
                                         start=(dvt == 0), stop=(dvt == 3))
                    wv = cp.tile([128, D], F32R, tag=f"wv2p{b}")
                    nc.vector.tensor_add(wv[:], wv_ps[:], bias8[:])
                    wv2p.append(wv)

            # ---------------- main loop ----------------
            with (
                tc.tile_pool(name="m_x", bufs=bufs_x) as mp_x,
                tc.tile_pool(name="m_ys", bufs=bufs_ysb) as mp_y,
                tc.tile_pool(name="m_sm", bufs=bufs_sm) as mp_s,
                # zb/y first: they alias the (closed) preamble PSUM banks and
                # are used late in each chunk, so chunk-0 transposes (xt) get
                # fresh banks and need not wait for the preamble to drain
                tc.tile_pool(name="ps_zb", bufs=bufs_zb, space="PSUM") as ps_zb,
                tc.tile_pool(name="ps_y", bufs=bufs_y, space="PSUM") as ps_y,
                tc.tile_pool(name="ps_xt", bufs=bufs_xt, space="PSUM") as ps_xt,
                tc.tile_pool(name="ps_s", bufs=bufs_s, space="PSUM") as ps_s,
            ):
                from contextlib import nullcontext
                rep_ctx = tc.For_i(0, repeat, 1) if repeat > 1 else nullcontext()
                with rep_ctx:
                    for b in range(B_PER_CORE):
                        for ci in range(CHUNKS_PER_B):
                            n0 = ci * CHUNK
                            x_src = x_d[b, n0:n0 + CHUNK, :].rearrange(
                                "(g p) k -> p g k", p=128)
                            x_sb = mp_x.tile([128, 4, D], F32, tag="x_sb")
                            if "load" not in skip:
                                nc.sync.dma_start(x_sb[:, 0:2, :], x_src[:, 0:2, :])
                                nc.sync.dma_start(x_sb[:, 2:4, :], x_src[:, 2:4, :])
                            else:
                                nc.vector.memset(x_sb[:, 0, 0:4], 0.0)

                            # transpose x -> x^T tiles [k-tile][128, 512] f32r
                            xT = mp_x.tile([128, 4, CHUNK], F32R, tag="xT")
                            if "transpose" not in skip:
                                for kt in range(4):
                                    xt_ps = ps_xt.tile([128, CHUNK], F32, tag="xt_ps")
                                    for g in range(4):
                                        nc.tensor.transpose(
                                            xt_ps[:, g * 128:(g + 1) * 128],
                                            x_sb[:, g, kt * 128:(kt + 1) * 128],
                                            ident[:])
                                    if kt % 2 == 0:
                                        nc.vector.tensor_copy(xT[:, kt, :], xt_ps[:])
                                    else:
                                        nc.scalar.copy(xT[:, kt, :], xt_ps[:])
                            else:
                                nc.vector.tensor_copy(xT[:, 0, 0:4], x_sb[:, 0, 0:4])

                            # S^T = W_s @ x^T  [128 ch, 512 n]
                            s_ps = ps_s.tile([128, CHUNK], F32, tag="s_ps")
                            for kt in range(4):
                                nc.tensor.matmul(s_ps[:], wsT[b][:, kt, :],
                                                 xT[:, kt, :],
                                                 start=(kt == 0), stop=(kt == 3))

                            # E = exp(S + mask_bias)
                            e_r = mp_s.tile([128, CHUNK], F32R, tag="e_r")
                            nc.scalar.activation(e_r[:], s_ps[:],
                                                 mybir.ActivationFunctionType.Exp,
                                                 bias=maskb[b][:], scale=1.0)

                            a_r = mp_s.tile([128, CHUNK], F32R, tag="a_r")
                            if "softmax" not in skip:
                                # Zrep[ch, n] = per-head sum of E, replicated
                                zb_ps = ps_zb.tile([128, CHUNK], F32, tag="zb_ps")
                                nc.tensor.matmul(zb_ps[:], blk16_r[:], e_r[:],
                                                 start=True, stop=True)
                                rzb = mp_s.tile([128, CHUNK], F32, tag="rzb")
                                nc.vector.reciprocal_approx_fast(rzb[:], zb_ps[:])
                                # A = E * recip(Zrep)  (normalized attention, f32r)
                                nc.vector.tensor_mul(a_r[:], e_r[:], rzb[:])
                            else:
                                nc.vector.tensor_copy(a_r[:], e_r[:])

                            # y[n-sub g] = A[:, g].T @ W_v2p  -> [128 n, 512 dout]
                            y_sb = mp_y.tile([128, 4, D], F32, tag="y_sb")
                            for g in range(4):
                                y_ps = ps_y.tile([128, D], F32, tag="y_ps")
                                nc.tensor.matmul(y_ps[:],
                                                 a_r[:, g * 128:(g + 1) * 128],
                                                 wv2p[b][:],
                                                 start=True, stop=True)
                                if g % 2 == 0:
                                    nc.scalar.copy(y_sb[:, g, :], y_ps[:])
                                else:
                                    nc.vector.tensor_copy(y_sb[:, g, :], y_ps[:])

                            if "store" not in skip:
                                y_dst = y_d[b, n0:n0 + CHUNK, :].rearrange(
                                    "(g p) k -> p g k", p=128)
                                nc.scalar.dma_start(y_dst, y_sb[:])

    nc.compile()
    return nc


def _prep_inputs(x, conditions, condition_mask, w_q, w_kv, w_proj, b_proj):
    """Host-side marshalling: shard over B, transpose/scale small weights."""
    x = np.ascontiguousarray(x, dtype=np.float32)
    conditions = np.asarray(conditions, dtype=np.float32)
    condition_mask = np.asarray(condition_mask)
    w_q = np.asarray(w_q, dtype=np.float32)
    w_kv = np.asarray(w_kv, dtype=np.float32)
    w_proj = np.asarray(w_proj, dtype=np.float32)
    b_proj = np.asarray(b_proj, dtype=np.float32)

    wq_scaled = np.ascontiguousarray(w_q * SCALE)            # [dq, ki]
    wkvT = np.ascontiguousarray(w_kv.T)                      # [256, 1024]
    wpT = np.ascontiguousarray(w_proj.T)                     # [dv, dout]
    bias8_rep = np.ascontiguousarray(
        np.tile((b_proj / H)[None, :], (128, 1)).astype(np.float32))
    blk16 = np.zeros((128, 128), dtype=np.float32)
    for h in range(H):
        blk16[h * C:(h + 1) * C, h * C:(h + 1) * C] = 1.0

    in_maps = []
    for core in range(N_CORES):
        b0 = core * B_PER_CORE
        condT = np.ascontiguousarray(
            np.transpose(conditions[b0:b0 + B_PER_CORE], (0, 2, 1)))
        mb = np.zeros((B_PER_CORE, 128, 1), dtype=np.float32)
        for b in range(B_PER_CORE):
            m = condition_mask[b0 + b].astype(bool)          # [16]
            col = np.where(np.tile(m, H), 0.0, NEG).astype(np.float32)
            mb[b, :, 0] = col
        in_maps.append(dict(
            x=np.ascontiguousarray(x[b0:b0 + B_PER_CORE]),
            condT=condT,
            wkvT=wkvT,
            wq_scaled=wq_scaled,
            wpT=wpT,
            bias8_rep=bias8_rep,
            mask_bias=mb,
            blk16=blk16,
        ))
    return in_maps


def kernel(x, conditions, condition_mask, w_q, w_kv, w_proj, b_proj):
    repeat = int(os.environ.get("MCCA_REPEAT", "1"))
    key = ("nc", repeat)
    if key not in _cache:
        _cache[key] = _build(repeat=repeat)
    nc = _cache[key]
    in_maps = _prep_inputs(x, conditions, condition_mask, w_q, w_kv,
                           w_proj, b_proj)
    res = run_bass_kernel_spmd(nc, in_maps, core_ids=list(range(N_CORES)))
    y = np.concatenate([r["y"] for r in res.results], axis=0)  # [16, 8192, 512]
    return np.ascontiguousarray(y.astype(np.float32))
